# revision 69
# baseline (speedup 1.0000x reference)
import numpy as np
import ml_dtypes
import concourse.bass as bass
import concourse.mybir as mybir
import concourse.tile as tile
from concourse import bacc
from concourse.bass_utils import run_bass_kernel_spmd
from concourse.alu_op_type import AluOpType

B, S, D = 4, 2048, 768
HPC = 6            # heads per core
PAIRS = 3
THETA = 10000.0
N_CORES = 8
F32 = mybir.dt.float32
BF16 = mybir.dt.bfloat16
BF = ml_dtypes.bfloat16
VW = HPC * 65      # 390: per-tb V tile width (6 heads x (64 dims + ones col))
EXP = mybir.ActivationFunctionType.Exp

_NC = None

CFG = {
    "w_attn": 6, "w0": 1, "w1": 2, "wf": 1, "drain_act": (0,),     # attention-chain weight in the merged last weave
}


def build_nc(with_collective=True):
    nc = bacc.Bacc("TRN2", target_bir_lowering=False, debug=False,
                   num_devices=N_CORES)
    xd = nc.dram_tensor("xd", [128, 4 * 3072], BF16, kind="ExternalInput")
    wqd = nc.dram_tensor("wqd", [128, 2304], BF16, kind="ExternalInput")
    wkd = nc.dram_tensor("wkd", [128, 2304], BF16, kind="ExternalInput")
    wvd = nc.dram_tensor("wvd", [128, 6 * VW], BF16, kind="ExternalInput")
    wod = nc.dram_tensor("wod", [128, 2304], BF16, kind="ExternalInput")
    cosd = nc.dram_tensor("cos", [128, S], BF16, kind="ExternalInput")
    sind = nc.dram_tensor("sin", [128, S], BF16, kind="ExternalInput")
    maskd = nc.dram_tensor("mask", [128, 128], BF16, kind="ExternalInput")
    eyed = nc.dram_tensor("eye", [128, 128], BF16, kind="ExternalInput")
    out = nc.dram_tensor("out", [S, D], F32, kind="ExternalOutput")

    with tile.TileContext(nc) as tc:
        with tc.tile_pool(name="persist", bufs=1) as pp, \
             tc.tile_pool(name="dram", bufs=1, space="DRAM") as dpool, \
             tc.tile_pool(name="trawp", bufs=4) as trawp, \
             tc.tile_pool(name="uhp", bufs=2) as uhp, \
             tc.tile_pool(name="swp", bufs=2) as swp, \
             tc.tile_pool(name="etp", bufs=12) as etp, \
             tc.tile_pool(name="rcpp", bufs=8) as rcpp, \
             tc.tile_pool(name="obp", bufs=3) as obp, \
             tc.tile_pool(name="pgen", bufs=2, space="PSUM") as pgen, \
             tc.tile_pool(name="pwide", bufs=2, space="PSUM") as pwide, \
             tc.tile_pool(name="pctx", bufs=2, space="PSUM") as pctxp:

            sb_q = [pp.tile([128, S], BF16, name=f"sb_q{i}") for i in range(PAIRS)]
            sb_k = [pp.tile([128, S], BF16, name=f"sb_k{i}") for i in range(PAIRS)]
            sb_v = pp.tile([128, 16 * VW], BF16)
            sb_ctx = [pp.tile([128, S], BF16, name=f"sb_ctx{i}") for i in range(PAIRS)]
            # normalized per-token context, [tok, 6*64] per token block
            ctx_sb = [pp.tile([128, 384], BF16, name=f"ctxs{i}") for i in range(16)]
            xcol = [None] + [pp.tile([128, 3072], BF16, name=f"xcol{i}")
                             for i in range(1, 4)]
            # token block 0 split in two tiles so the first chains depend
            # only on the first, small DMA (DMA-write deps are per-tile)
            xc0a = pp.tile([128, 1024], BF16)
            xc0b = pp.tile([128, 2048], BF16)
            # per-pair weight tiles (pr-major host layout) for the same reason
            sb_wq = [pp.tile([128, 768], BF16, name=f"sb_wq{i}") for i in range(PAIRS)]
            sb_wk = [pp.tile([128, 768], BF16, name=f"sb_wk{i}") for i in range(PAIRS)]
            sb_wv = pp.tile([128, 6 * VW], BF16)
            sb_wo = pp.tile([128, 2304], BF16)
            sb_cos = pp.tile([128, S], BF16)
            sb_sin = pp.tile([128, S], BF16)
            sb_mask = pp.tile([128, 128], BF16)
            sb_eye = pp.tile([128, 128], BF16)
            bounce_in = dpool.tile([S, D], F32)
            bounce_out = dpool.tile([S, D], F32)

            def gen_tile():
                return pgen.tile([128, 512], F32, name="pgen_t")

            def wide_tile():
                return pwide.tile([128, 1024], F32, name="pw_t")

            # warm up the PE p-state before real work arrives
            warm = pp.tile([128, 512], BF16)
            nc.vector.memset(warm[:], 1.0)
            # input loads, most-urgent first
            nc.sync.dma_start(sb_wq[0][:], wqd[:, 0:768])
            nc.sync.dma_start(xc0a[:], xd[:, 0:1024])
            nc.sync.dma_start(sb_cos[:, 0:512], cosd[:, 0:512])
            nc.sync.dma_start(sb_sin[:, 0:512], sind[:, 0:512])
            nc.sync.dma_start(xc0b[:], xd[:, 1024:3072])
            nc.sync.dma_start(sb_wq[1][:], wqd[:, 768:1536])
            nc.sync.dma_start(sb_wq[2][:], wqd[:, 1536:2304])
            nc.sync.dma_start(sb_wv[:], wvd[:])
            nc.sync.dma_start(sb_wk[0][:], wkd[:, 0:768])
            nc.sync.dma_start(sb_wk[1][:], wkd[:, 768:1536])
            nc.sync.dma_start(sb_wk[2][:], wkd[:, 1536:2304])
            nc.sync.dma_start(sb_cos[:, 512:2048], cosd[:, 512:2048])
            nc.sync.dma_start(sb_sin[:, 512:2048], sind[:, 512:2048])
            nc.sync.dma_start(sb_mask[:], maskd[:])
            nc.sync.dma_start(sb_eye[:], eyed[:])
            wt = gen_tile()
            for i in range(7):
                nc.tensor.matmul(wt[:], warm[:, 0:128], warm[:],
                                 start=(i == 0), stop=(i == 6))
            with nc.allow_low_precision(reason="warmup drain"):
                nc.vector.tensor_copy(warm[:], wt[:])

            def proj_tt(tt):
                """QKV projection + RoPE for token block tt (512 tokens).

                Q pairs first so the next q-block's attention unblocks early;
                one DVE drain copy per chain frees PSUM fast and enables 2x
                bf16 DVE modes for the rope muls; rope adds go to Pool."""
                xc = xcol[tt]
                csl = sb_cos[:, tt * 512:(tt + 1) * 512]
                ssl = sb_sin[:, tt * 512:(tt + 1) * 512]
                uh = uhp.tile([128, 3072], BF16)
                swf = swp.tile([128, 3072], BF16)

                def xsl(ck, off, width):
                    base = ck * 512 + off
                    if tt > 0:
                        return xc[:, base:base + width]
                    if base < 1024:
                        return xc0a[:, base:base + width]
                    return xc0b[:, base - 1024:base - 1024 + width]

                def qk_pair(wi, wsb, dst, pr):
                    if tt == 0:
                        pcw = wide_tile()
                        pc = pcw[:, 0:512]
                    else:
                        pc = gen_tile()
                    for ck in range(6):
                        nc.tensor.matmul(
                            pc[:],
                            wsb[pr][:, ck * 128:(ck + 1) * 128],
                            xsl(ck, 0, 512),
                            start=(ck == 0), stop=(ck == 5))
                    traw = trawp.tile([128, 512], BF16)
                    with nc.allow_low_precision(reason="bf16 qk"):
                        if tt in CFG["drain_act"]:
                            nc.scalar.copy(traw[:], pc[:])
                        else:
                            nc.vector.tensor_copy(traw[:], pc[:])
                        nc.vector.tensor_mul(
                            dst[pr][:, tt * 512:(tt + 1) * 512], traw[:], csl)
                        nc.vector.tensor_mul(
                            uh[:, (wi * 3 + pr) * 512:(wi * 3 + pr + 1) * 512],
                            traw[:], ssl)

                def v_block(tj):
                    tb = tt * 4 + tj
                    pvt = gen_tile()
                    for ck in range(6):
                        nc.tensor.matmul(
                            pvt[:, 0:VW],
                            xsl(ck, tj * 128, 128),
                            sb_wv[:, ck * VW:(ck + 1) * VW],
                            start=(ck == 0), stop=(ck == 5))
                    with nc.allow_low_precision(reason="bf16 v"):
                        nc.scalar.copy(sb_v[:, tb * VW:(tb + 1) * VW],
                                       pvt[:, 0:VW])
                    # ones AFTER the copy (the copy clobbers these columns)
                    nc.vector.memset(
                        sb_v[:, tb * VW + 64:(tb + 1) * VW:65], 1.0)

                def swap_add(wi, dst, pr):
                    # rope pair swap: within each 32-partition block the
                    # even/odd halves are interleaved at 16, so the swap is
                    # an in-block shuffle i^16 on the DVE crossbar
                    sl = slice((wi * 3 + pr) * 512, (wi * 3 + pr + 1) * 512)
                    nc.vector.stream_shuffle(
                        swf[:, sl], uh[:, sl], [i ^ 16 for i in range(32)])
                    d = dst[pr][:, tt * 512:(tt + 1) * 512]
                    with nc.allow_low_precision(reason="bf16 qk add"):
                        nc.gpsimd.tensor_add(d, d, swf[:, sl])

                # Q first (unblocks the next q-block's attention), then V
                # (so a chained attn3 reaches its diagonal only after the V
                # blocks are emitted), then K
                for pr in range(PAIRS):
                    qk_pair(1, sb_wq, sb_q, pr)
                    swap_add(1, sb_q, pr)
                    yield
                for tj in range(4):
                    v_block(tj)
                    yield
                for pr in range(PAIRS):
                    qk_pair(0, sb_wk, sb_k, pr)
                    swap_add(0, sb_k, pr)
                    yield

            def finish_tb(tb):
                """Transpose ctx to [hd, tok] on the PE, output projection,
                store. Must be emitted only after every finalize that writes
                ctx_sb[tb] (tile deps only order readers emitted later).

                PSUM drains go to Act for the early fins (they execute while
                Act still has headroom) and to DVE for the late ones (the
                attn3 stretch saturates Act but leaves DVE mostly idle)."""
                dst = bounce_in if with_collective else out
                early = False
                for c in range(PAIRS):
                    ptr = gen_tile()
                    nc.tensor.matmul(
                        ptr[:, 0:128],
                        ctx_sb[tb][:, c * 128:(c + 1) * 128],
                        sb_eye[:],
                        start=True, stop=True)
                    with nc.allow_low_precision(reason="bf16 ctxT"):
                        if early:
                            nc.scalar.copy(
                                sb_ctx[c][:, tb * 128:(tb + 1) * 128],
                                ptr[:, 0:128])
                        else:
                            nc.vector.tensor_copy(
                                sb_ctx[c][:, tb * 128:(tb + 1) * 128],
                                ptr[:, 0:128])
                ob = obp.tile([128, D], F32)
                for nn in range(2):
                    pot = gen_tile()
                    for ci in range(PAIRS):
                        nc.tensor.matmul(
                            pot[:, 0:384],
                            sb_ctx[ci][:, tb * 128:(tb + 1) * 128],
                            sb_wo[:, ci * 768 + nn * 384:
                                     ci * 768 + nn * 384 + 384],
                            start=(ci == 0), stop=(ci == 2))
                    if early:
                        nc.scalar.copy(ob[:, nn * 384:(nn + 1) * 384],
                                       pot[:, 0:384])
                    else:
                        nc.vector.tensor_copy(
                            ob[:, nn * 384:(nn + 1) * 384], pot[:, 0:384])
                nc.gpsimd.dma_start(dst[tb * 128:(tb + 1) * 128, :], ob[:])

            def attn_qt(qt):
                """Full causal attention for q-block qt (512 queries), all 6
                heads, flat one-stage software pipeline across heads."""
                fulls = 4 * qt
                pctx_h = {}

                def full_score(h, kb2):
                    pr, off = h // 2, (h % 2) * 64
                    psw = wide_tile()
                    for s in (0, 1):
                        kb = kb2 + s
                        nc.tensor.matmul(
                            psw[:, s * 512:(s + 1) * 512],
                            sb_k[pr][off:off + 64, kb * 128:(kb + 1) * 128],
                            sb_q[pr][off:off + 64, qt * 512:(qt + 1) * 512],
                            start=True, stop=True)
                    etw = etp.tile([128, 1024], BF16)
                    with nc.allow_low_precision(reason="bf16 probs"):
                        nc.scalar.activation(etw[:], psw[:], EXP)
                    return etw

                def full_ctx(h, kb2, etw):
                    pctx = pctx_h[h]
                    for s in (0, 1):
                        kb = kb2 + s
                        for qj in range(4):
                            nc.tensor.matmul(
                                pctx[:, qj * 65:qj * 65 + 65],
                                etw[:, s * 512 + qj * 128:
                                       s * 512 + (qj + 1) * 128],
                                sb_v[:, kb * VW + h * 65:
                                        kb * VW + h * 65 + 65],
                                start=(kb == 0 and qj == 0), stop=False,
                                skip_group_check=True)

                # diag part A: blocks j=0 (cols 0:512) and j=1 (cols 512:896)
                # merged into one PSUM tile -> single exp
                def diagA_score(h):
                    pr, off = h // 2, (h % 2) * 64
                    psw = wide_tile()
                    nc.tensor.matmul(
                        psw[:, 0:512],
                        sb_k[pr][off:off + 64,
                                 (fulls + 0) * 128:(fulls + 1) * 128],
                        sb_q[pr][off:off + 64, qt * 512:(qt + 1) * 512],
                        start=True, stop=True)
                    nc.tensor.matmul(
                        psw[:, 512:896],
                        sb_k[pr][off:off + 64,
                                 (fulls + 1) * 128:(fulls + 2) * 128],
                        sb_q[pr][off:off + 64,
                                 qt * 512 + 128:(qt + 1) * 512],
                        start=True, stop=True)
                    etd = etp.tile([128, 1024], BF16)
                    with nc.allow_low_precision(reason="bf16 probs"):
                        nc.scalar.activation(etd[:, 0:896], psw[:, 0:896], EXP)
                    meng = nc.gpsimd if qt < 2 else nc.vector
                    with nc.allow_low_precision(reason="bf16 mask"):
                        meng.tensor_mul(etd[:, 0:128],
                                        etd[:, 0:128], sb_mask[:])
                        meng.tensor_mul(etd[:, 512:640],
                                        etd[:, 512:640], sb_mask[:])
                    return etd

                # diag part B: blocks j=2 (cols 0:256) and j=3 (cols 256:384)
                def diagB_score(h):
                    pr, off = h // 2, (h % 2) * 64
                    psw = wide_tile()
                    nc.tensor.matmul(
                        psw[:, 0:256],
                        sb_k[pr][off:off + 64,
                                 (fulls + 2) * 128:(fulls + 3) * 128],
                        sb_q[pr][off:off + 64,
                                 qt * 512 + 256:(qt + 1) * 512],
                        start=True, stop=True)
                    nc.tensor.matmul(
                        psw[:, 256:384],
                        sb_k[pr][off:off + 64,
                                 (fulls + 3) * 128:(fulls + 4) * 128],
                        sb_q[pr][off:off + 64,
                                 qt * 512 + 384:(qt + 1) * 512],
                        start=True, stop=True)
                    etd = etp.tile([128, 512], BF16)
                    with nc.allow_low_precision(reason="bf16 probs"):
                        nc.scalar.activation(etd[:, 0:384], psw[:, 0:384], EXP)
                    meng = nc.gpsimd if qt < 2 else nc.vector
                    with nc.allow_low_precision(reason="bf16 mask"):
                        meng.tensor_mul(etd[:, 0:128],
                                        etd[:, 0:128], sb_mask[:])
                        meng.tensor_mul(etd[:, 256:384],
                                        etd[:, 256:384], sb_mask[:])
                    return etd

                def finalize(h, qj, pctx):
                    rcp = rcpp.tile([128, 1], F32)
                    nc.vector.reciprocal(
                        rcp[:], pctx[:, qj * 65 + 64:qj * 65 + 65])
                    with nc.allow_low_precision(reason="bf16 ctx"):
                        nc.vector.tensor_scalar(
                            ctx_sb[4 * qt + qj][:, h * 64:(h + 1) * 64],
                            pctx[:, qj * 65:qj * 65 + 64],
                            rcp[:], None, AluOpType.mult)

                def diagA_ctx(h, etd):
                    pctx = pctx_h[h]
                    for qj in range(4):       # j=0 block
                        kb = fulls
                        nc.tensor.matmul(
                            pctx[:, qj * 65:qj * 65 + 65],
                            etd[:, qj * 128:(qj + 1) * 128],
                            sb_v[:, kb * VW + h * 65:kb * VW + h * 65 + 65],
                            start=(kb == 0 and qj == 0), stop=(qj == 0),
                            skip_group_check=True)
                        if qj == 0:
                            finalize(h, 0, pctx)
                    for qj in range(1, 4):    # j=1 block
                        kb = fulls + 1
                        nc.tensor.matmul(
                            pctx[:, qj * 65:qj * 65 + 65],
                            etd[:, 512 + (qj - 1) * 128:512 + qj * 128],
                            sb_v[:, kb * VW + h * 65:kb * VW + h * 65 + 65],
                            start=False, stop=(qj == 1),
                            skip_group_check=True)
                        if qj == 1:
                            finalize(h, 1, pctx)

                def diagB_ctx(h, etd):
                    pctx = pctx_h[h]
                    for qj in range(2, 4):    # j=2 block
                        kb = fulls + 2
                        nc.tensor.matmul(
                            pctx[:, qj * 65:qj * 65 + 65],
                            etd[:, (qj - 2) * 128:(qj - 1) * 128],
                            sb_v[:, kb * VW + h * 65:kb * VW + h * 65 + 65],
                            start=False, stop=(qj == 2),
                            skip_group_check=True)
                        if qj == 2:
                            finalize(h, 2, pctx)
                    kb = fulls + 3            # j=3 block
                    nc.tensor.matmul(
                        pctx[:, 3 * 65:3 * 65 + 65],
                        etd[:, 256:384],
                        sb_v[:, kb * VW + h * 65:kb * VW + h * 65 + 65],
                        start=False, stop=True,
                        skip_group_check=True)
                    finalize(h, 3, pctx)

                def get_pctx(h):
                    if h not in pctx_h:
                        pctx_h[h] = pctxp.tile([128, 260], F32, name="pctx_t")
                    return pctx_h[h]

                stages = []
                for h in range(HPC):
                    for kb2 in range(0, fulls, 2):
                        stages.append(
                            (lambda h=h, kb2=kb2: full_score(h, kb2),
                             lambda et, h=h, kb2=kb2:
                                 (get_pctx(h), full_ctx(h, kb2, et))))
                    stages.append((lambda h=h: diagA_score(h),
                                   lambda et, h=h:
                                       (get_pctx(h), diagA_ctx(h, et))))
                    stages.append((lambda h=h: diagB_score(h),
                                   lambda et, h=h:
                                       (get_pctx(h), diagB_ctx(h, et))))
                # one-step software pipeline across all heads of this qt
                prev = None
                for sc, cx in stages:
                    et = sc()
                    if prev is not None:
                        prev[1](prev[0])
                    prev = (et, cx)
                    yield
                prev[1](prev[0])
                yield

            def fin_range(tbs):
                for tb in tbs:
                    finish_tb(tb)
                    yield

            def delayed(g, n):
                for _ in range(n):
                    yield
                yield from g

            def chain(*gens):
                for g in gens:
                    yield from g

            def weave(gens):
                # gens: list of generators or (generator, weight)
                gw = [(g, 1) if not isinstance(g, tuple) else g for g in gens]
                while gw:
                    alive = []
                    for g, w in gw:
                        done = False
                        for _ in range(w):
                            try:
                                next(g)
                            except StopIteration:
                                done = True
                                break
                        if not done:
                            alive.append((g, w))
                    gw = alive

            weave([proj_tt(0)])
            nc.sync.dma_start(xcol[1][:], xd[:, 3072:6144])
            nc.sync.dma_start(sb_wo[:], wod[:])
            nc.sync.dma_start(xcol[2][:], xd[:, 6144:9216])
            nc.sync.dma_start(xcol[3][:], xd[:, 9216:12288])
            weave([(proj_tt(1), 1), (attn_qt(0), CFG["w0"])])
            weave([(proj_tt(2), 1), (attn_qt(1), CFG["w1"])])
            # Last phase: attn2 chains into attn3 and then the qt3 fins in
            # ONE stream — emission order (which is what tile dependency
            # tracking keys on) provably places every qt3 fin after all of
            # attn3's finalizes. Fins for qt0..qt2 run in a parallel stream:
            # qt0/qt1 completed phases ago; the qt2 fins (8..11) are reached
            # only at rounds >= 9, after attn2's last finalizes have been
            # emitted earlier in the same rounds.
            if CFG["wf"] == 1:
                fin_stream = fin_range(range(0, 12))
            else:
                # at weight wf the qt2 fins (8..11) must still be emitted at
                # rounds >= 8 (attn2's last finalizes flush at round 7)
                fin_stream = chain(fin_range(range(0, 8)),
                                   delayed(fin_range(range(8, 12)),
                                           8 * CFG["wf"] - 9))
            weave([(proj_tt(3), 1),
                   (chain(attn_qt(2), attn_qt(3),
                          fin_range(range(12, 16))), CFG["w_attn"]),
                   (fin_stream, CFG["wf"])])

            if with_collective:
                nc.gpsimd.collective_compute(
                    "AllReduce", mybir.AluOpType.add,
                    replica_groups=[[0, 1], [2, 3], [4, 5], [6, 7]],
                    ins=[bounce_in.opt()], outs=[bounce_out.opt()])
                nc.sync.dma_start(out[:], bounce_out[:])
    nc.compile()
    return nc


def make_in_maps(x, w_q, w_k, w_v, w_o, token_positions):
    xn = np.asarray(x, np.float32)
    wqn = np.asarray(w_q, np.float32)
    wkn = np.asarray(w_k, np.float32)
    wvn = np.asarray(w_v, np.float32)
    won = np.asarray(w_o, np.float32)
    pos = np.asarray(token_positions).astype(np.float32)
    inv = THETA ** (-np.arange(32, dtype=np.float32) / 32.0)
    ang = inv[:, None] * pos[None, :]
    c32 = np.cos(ang).astype(np.float32)
    s32 = np.sin(ang).astype(np.float32)
    cblock = np.concatenate([c32[:16], c32[:16], c32[16:], c32[16:]], axis=0)
    sblock = np.concatenate([s32[:16], -s32[:16], s32[16:], -s32[16:]], axis=0)
    cosd = np.tile(cblock, (2, 1)).astype(BF)
    sind = np.tile(sblock, (2, 1)).astype(BF)
    maskd = (np.arange(128)[:, None] <= np.arange(128)[None, :]).astype(BF)
    eyed = np.eye(128, dtype=np.float32).astype(BF)
    perm_eo = np.r_[0:32:2, 1:32:2, 32:64:2, 33:64:2]
    in_maps = []
    for c in range(N_CORES):
        b, hg = c // 2, c % 2
        heads = hg * HPC + np.arange(HPC)
        rows_eo = (heads[:, None] * 64 + perm_eo[None, :]).reshape(-1)
        # x: xd[p, tt*3072 + ck*512 + s] = x[b, tt*512+s, ck*128+p]
        xd_ = (xn[b].reshape(4, 512, 6, 128).transpose(3, 0, 2, 1)
               .reshape(128, 4 * 3072)).astype(BF)
        # wq/wk pr-major: w*d[p, pr*768 + ck*128 + jj] = w_perm[pr*128+jj, ck*128+p]
        wql = wqn[rows_eo] * 0.125
        wqd_ = (wql.reshape(3, 128, 6, 128).transpose(3, 0, 2, 1)
                .reshape(128, 2304)).astype(BF)
        wkl = wkn[rows_eo]
        wkd_ = (wkl.reshape(3, 128, 6, 128).transpose(3, 0, 2, 1)
                .reshape(128, 2304)).astype(BF)
        # wv: wvd[p, ck*390 + h*65 + jj] = wv[(hg*6+h)*64 + jj, ck*128+p]
        wvl = np.zeros((VW, D), np.float32)
        for h in range(HPC):
            g = hg * HPC + h
            wvl[h * 65:h * 65 + 64] = wvn[g * 64:(g + 1) * 64]
        wvd_ = (wvl.reshape(VW, 6, 128).transpose(2, 1, 0)
                .reshape(128, 6 * VW)).astype(BF)
        # wo: wod[p, ci*768 + od] = w_o[od, hg*384 + ci*128 + p]
        wol = won[:, hg * 384:(hg + 1) * 384]
        wod_ = (wol.T.reshape(3, 128, 768).transpose(1, 0, 2)
                .reshape(128, 2304)).astype(BF)
        in_maps.append({
            "xd": xd_, "wqd": wqd_, "wkd": wkd_, "wvd": wvd_, "wod": wod_,
            "cos": cosd, "sin": sind, "mask": maskd, "eye": eyed,
        })
    return in_maps


def kernel(x, w_q, w_k, w_v, w_o, token_positions):
    global _NC
    if _NC is None:
        _NC = build_nc()
    in_maps = make_in_maps(x, w_q, w_k, w_v, w_o, token_positions)
    res = run_bass_kernel_spmd(_NC, in_maps, core_ids=list(range(N_CORES)))
    return np.stack([res.results[2 * b]["out"] for b in range(B)], axis=0)
